# revision 13
# baseline (speedup 1.0000x reference)
"""Distance-correlation (DcorLoss) kernel for 8 trn2 NeuronCores.

Math: for x, y [n=8192, d=128]:
  a = pairwise_dist(x), b = pairwise_dist(y)   (n x n, symmetric, zero diag)
  A = double_center(a), B = double_center(b)
  dcor = -sqrt(sum(A*B)) / sqrt(sqrt(sum(A*A)) * sqrt(sum(B*B)))

Never materialize A/B:
  sum(A o B) = sum(a o b) - 2/n dot(rs_a, rs_b) + sum(a) sum(b) / n^2
and the squared-distance Frobenius norms have a closed form (host, exact):
  sum_ij dist^2 = 2n sum_i |x_i|^2 - 2 |sum_i x_i|^2
so the device only streams: row sums of a and b, column sums (PE matmul
with ones weights), and sum (a - mu) * b (DVE accum). All combining is
host fp64.

Structural tricks (all in the single "symdr" program):

1. fp8 DoubleRow matmul (perf_mode=DoubleRow, K=256 virtual): plane 0
   carries the 128 data rows (-2 x_blk^T x gram), plane 1 rows 0..2 carry
   the column-norm hi/lo/lo2 splits against all-ones weight rows. One MM
   per 512-col psum half computes n_i-free sq distances entirely.
2. Symmetry: core c computes only local windows 0..4 (its diagonal block
   + 4 cyclic neighbors) = 5/8 of the row-block work. Full-matrix sums
   use sum_full = 2*sum_computed - S(w=0) - S(w=4); full row sums add
   mirrored per-column sums of windows 1..3 (PE ones-matmuls into one
   psum bank at partitions 0/32/64/96), gathered on host.
3. Per-core COLUMN ROTATION: core c's column j is global (j + c*1024)
   mod n, so the diagonal lands in window 0 on every core and the SPMD
   program is identical; the mu^2 diagonal forcing (sqrt NaN-safety)
   costs 2 fp8 matmuls on window-0 tiles only.
4. Row-sum accumulation split across engines: windows 1..3 get their
   per-row sums from the bf16 cast pass (DVE tensor_scalar accum_out,
   fp32 internal, reads the PRE-rounding fp32 sqrt outputs); windows
   0/4 keep the ACT accum_out. Keeps the ACT pitch minimal on 48 of 80
   sqrt passes; DVE accumulator reads are ~6x cheaper than ACT's.
5. DMA compaction: moving tensors ship plane-0 only ([P, 5W] fp8) plus a
   [3, 5W] column-norm strip; plane-1 rows 3..127 are zeroed on-device
   (uint32-bitcast DVE memsets; required because weights there are 0.0
   and fp8 garbage could be NaN, 0*NaN=NaN). Weight plane-1 (3 ones rows
   + zeros) is memset on gpsimd. The 512KB eyewide diag-fix tensor is
   built on-device from the 16KB fp8 eye. Startup DMAs are ordered so
   the first matmul gates on ~100KB (ci-0 weight slice + first half
   window) instead of ~1MB.

Precision: the final sums cancel ~1e8 -> ~1e6, which amplifies any BIAS
~1e4x. bf16-rounded sqrt outputs carry E[delta] ~ -2e-4 -> 5% error, so:
products and row-sum accums run on fp32 ACT outputs (unbiased); only the
column-sum matmuls read bf16 copies; global Ra/Rb come from the unbiased
fp32 accum totals; the dot products are mean-centered, which cancels the
constant per-row bias of the mirrored column sums.

ACT (ScalarE) is the bottleneck: 80 sqrt passes at 1 elem/cycle/lane;
PE (DR mains + colsums) and DVE (products + casts) overlap underneath.
"""

import numpy as np
import ml_dtypes

import concourse.bass as bass
import concourse.tile as tile
from concourse import bacc, mybir
from concourse.bass_utils import run_bass_kernel_spmd

P = 128            # partitions / d
N = 8192           # points
NCORES = 8
BLK = N // NCORES  # 1024 rows per core
CI_N = BLK // P    # 8 row chunks per core
W = 1024           # column window
JT_N = N // W      # 8 column windows
MU = 16.0          # ~E[pairwise dist] for randn d=128; any constant is exact
RES_W = 24

BF16 = ml_dtypes.bfloat16
F8 = ml_dtypes.float8_e4m3

DEFAULT_MODE = "symdr"
MODES = ("symdr",)
NW = 5             # local windows 0..4 (diag + 4 cyclic)
NWW = NW * W       # 5120 moving columns per core

_programs = {}


def _build(mode: str):
    dt = mybir.dt
    f32 = dt.float32
    bf = dt.bfloat16
    A = mybir.AluOpType
    AF = mybir.ActivationFunctionType
    f8 = dt.float8e4

    nc = bacc.Bacc("TRN2", target_bir_lowering=False, debug=False,
                   num_devices=NCORES)

    # moving tensors: plane-0 data only, window-major (5 windows)
    dxm = nc.dram_tensor("xm", [P, NW, W], f8, kind="ExternalInput").ap()
    dym = nc.dram_tensor("ym", [P, NW, W], f8, kind="ExternalInput").ap()
    # column-norm hi/lo/lo2 splits for the 5 windows
    dnx3 = nc.dram_tensor("nx3", [3, NWW], f8, kind="ExternalInput").ap()
    dny3 = nc.dram_tensor("ny3", [3, NWW], f8, kind="ExternalInput").ap()
    # weights: plane-0 (-2 xT block) only
    dwx = nc.dram_tensor("wx", [P, BLK], f8, kind="ExternalInput").ap()
    dwy = nc.dram_tensor("wy", [P, BLK], f8, kind="ExternalInput").ap()
    dnbx = nc.dram_tensor("nbx", [P, CI_N], f32, kind="ExternalInput").ap()
    dnby = nc.dram_tensor("nby", [P, CI_N], f32, kind="ExternalInput").ap()
    deye = nc.dram_tensor("eye128", [P, P], f8, kind="ExternalInput").ap()
    dout = nc.dram_tensor("out", [P, RES_W], f32, kind="ExternalOutput").ap()
    dcols = nc.dram_tensor("cols", [P, 3 * 512], f32,
                           kind="ExternalOutput").ap()
    dst01 = [nc.dram_tensor(f"st{q}o", [P, CI_N * NW], f32,
                            kind="ExternalOutput").ap() for q in range(2)]
    dst2 = nc.dram_tensor("st2", [P, CI_N * NW], f32,
                          kind="ExternalOutput").ap()

    with tile.TileContext(nc) as tc:
        with tc.tile_pool(name="const", bufs=1) as cp, \
             tc.tile_pool(name="psum", bufs=3, space="PSUM") as pp, \
             tc.tile_pool(name="cspsum", bufs=2, space="PSUM") as csp, \
             tc.tile_pool(name="ab", bufs=3) as abp, \
             tc.tile_pool(name="trd", bufs=2) as trd:

            # ── persistent operands ───────────────────────────────────
            # full 8-window plane stride (matmul moving AP reads plane0/1
            # pairs 8192B apart; 5120B stride measured slower) ; only the
            # first 5 windows are ever filled or read
            xm = cp.tile([P, 2, JT_N * W], f8, tag="xm")
            ym = cp.tile([P, 2, JT_N * W], f8, tag="ym")
            wx = cp.tile([P, 2, BLK], f8, tag="wx")
            wy = cp.tile([P, 2, BLK], f8, tag="wy")
            nbx = cp.tile([P, CI_N], f32, tag="nbx")
            nby = cp.tile([P, CI_N], f32, tag="nby")
            eye128 = cp.tile([P, P], f8, tag="eye128")
            eyew = cp.tile([P, 4 * 512], f8, tag="eyew")
            onesP = cp.tile([P, 1], bf, tag="onesP")
            colsave = cp.tile([P, 3 * 512], f32, tag="colsave")
            res = cp.tile([P, RES_W], f32, tag="res")
            st = [cp.tile([P, CI_N * NW], f32, tag=f"st{q}", name=f"st{q}")
                  for q in range(3)]
            # window-1 A-tiles are processed before any B-tile (x-side
            # data arrives first); their fp32 sqrt outputs persist here
            # until the B-phase product pass consumes them
            aW1 = cp.tile([P, CI_N * W], f32, tag="aW1")

            # ── on-device constant construction (overlaps the DMA wave)
            # table-load dummy operands first: the scalar-queue warmup
            # activation must not wait on later memsets
            wur = cp.tile([2, 512], bf, tag="wur")
            nc.vector.memset(wur[:], 0.0)
            tldu = cp.tile([1, 8], bf, tag="tldu")
            tlda = cp.tile([1, 1], f32, tag="tlda")
            tldb = cp.tile([1, 1], f32, tag="tldb")
            nc.vector.memset(tldb[:], 0.0)
            # moving plane-1 rows 3..127 must be exact zeros (weights are
            # 0.0 there; fp8 SBUF garbage could be NaN and 0*NaN=NaN).
            # Zero the full plane (uint32 bitcast); the norm-strip DMAs
            # below overwrite rows 0..2 (WAW-ordered by Tile, so the
            # memsets MUST be emitted before the dma_starts).
            nc.vector.memset(xm[:, 1, 0:NWW].bitcast(dt.uint32), 0)
            nc.vector.memset(ym[:, 1, 0:NWW].bitcast(dt.uint32), 0)
            # weight plane-1 = 3 ones rows + zeros (fp8 1.0 = 0x38)
            ONES_F8_U32 = 0x38383838
            nc.vector.memset(wx[:, 1, :].bitcast(dt.uint32), 0)
            nc.vector.memset(wy[:, 1, :].bitcast(dt.uint32), 0)
            nc.vector.memset(wx[0:3, 1, :].bitcast(dt.uint32), ONES_F8_U32)
            nc.vector.memset(wy[0:3, 1, :].bitcast(dt.uint32), ONES_F8_U32)
            nc.vector.memset(onesP[:], 1.0)
            nc.vector.memset(res[:], 0.0)
            nc.vector.memset(eyew[:].bitcast(dt.uint32), 0)

            # ── DMA wave ──────────────────────────────────────────────
            # gpsimd queue (idle): tiny operands that gate the first
            # matmul (norm strips) + the late diag eye
            nc.gpsimd.dma_start(xm[0:3, 1, 0:NWW], dnx3[:])
            nc.gpsimd.dma_start(ym[0:3, 1, 0:NWW], dny3[:])
            nc.gpsimd.dma_start(eye128[:], deye[:])
            # eyewide built from eye128: eye at column 640k, k=0..3
            for k in range(4):
                nc.vector.tensor_copy(eyew[:, 640 * k:640 * k + P],
                                      eye128[:])

            # window-1 x-side on sync, finest-first: the w1 A-tiles run
            # before any B-tile, so only x-data gates the first ACTIVATE;
            # y-side rides the scalar queue behind the table-load warmup.
            # In-flight transfers on one queue run concurrently on
            # separate DMA engines (~24GB/s each, ~0.65us per dispatch).
            nc.sync.dma_start(xm[0:64, 0, W:2 * W], dxm[0:64, 1])
            nc.sync.dma_start(xm[64:P, 0, W:2 * W], dxm[64:P, 1])
            nc.sync.dma_start(wx[:, 0, 0:256], dwx[:, 0:256])
            nc.sync.dma_start(nbx[:], dnbx[:])
            nc.sync.dma_start(wx[:, 0, 256:640], dwx[:, 256:640])
            nc.sync.dma_start(wx[:, 0, 640:BLK], dwx[:, 640:BLK])
            for w in (2, 3, 4, 0):
                nc.sync.dma_start(xm[:, 0, w * W:(w + 1) * W], dxm[:, w])
                nc.sync.dma_start(ym[:, 0, w * W:(w + 1) * W], dym[:, w])

            # scalar queue: sqrt table loads + warmup activation first
            # (~2.9us), then the y-side w1 operands (deadline ~20us)
            nc.scalar.activation(tldu[:], wur[0:1, 0:8], AF.Sqrt,
                                 bias=tldb[:], accum_out=tlda[:])
            nc.scalar.dma_start(ym[0:64, 0, W:2 * W], dym[0:64, 1])
            nc.scalar.dma_start(ym[64:P, 0, W:2 * W], dym[64:P, 1])
            nc.scalar.dma_start(wy[:, 0, 0:512], dwy[:, 0:512])
            nc.scalar.dma_start(wy[:, 0, 512:BLK], dwy[:, 512:BLK])
            nc.scalar.dma_start(nby[:], dnby[:])

            # ── main loop ─────────────────────────────────────────────
            # window 1 first (no diag fix -> shortest dependency chain to
            # the first ACTIVATE); diag window 0 last
            def mains(ps_, ww, mov, w, ci):
                h_diag = ci // 4
                for h in range(2):
                    last = not (w == 0 and h == h_diag)
                    nc.tensor.matmul(
                        ps_[:, bass.ds(h * 512, 512)],
                        ww[:, :, bass.ts(ci, P)],
                        mov[:, :, bass.ds(w * W + h * 512, 512)],
                        start=True, stop=last,
                        perf_mode=mybir.MatmulPerfMode.DoubleRow)
                if w == 0:
                    nc.tensor.matmul(
                        ps_[:, bass.ds(h_diag * 512, 512)],
                        eye128[:], eyew[:, bass.ts(ci % 4, 512)],
                        start=False, stop=True)

            def colsum(cs, t_, r, ci):
                for h in range(2):
                    nc.tensor.matmul(
                        cs[r + 32 * h:r + 32 * h + 1, 0:512],
                        onesP[:], t_[:, bass.ds(h * 512, 512)],
                        start=(ci == 0), stop=(ci == CI_N - 1),
                        tile_position=(0, r + 32 * h))

            for w in (1, 2, 3, 4, 0):
                do_cs = 1 <= w <= 3
                if do_cs:
                    cs = csp.tile([P, 512], f32, tag="cs")
                if w == 1:
                    # A-phase: only x-side data needed; y transfers hide
                    for ci in range(CI_N):
                        col = ci * NW + w
                        psA = pp.tile([P, W], f32, tag="ps")
                        mains(psA, wx, xm, w, ci)
                        aT = aW1[:, bass.ts(ci, W)]
                        nc.scalar.activation(aT, psA[:], AF.Sqrt,
                                             bias=nbx[:, ci:ci + 1],
                                             accum_out=st[0][:, col:col + 1])
                        a16 = trd.tile([P, W], bf, tag="a16")
                        nc.vector.tensor_copy(a16[:], aT)
                        colsum(cs, a16, 0, ci)
                    # B-phase
                    for ci in range(CI_N):
                        col = ci * NW + w
                        psB = pp.tile([P, W], f32, tag="ps")
                        mains(psB, wy, ym, w, ci)
                        bT = abp.tile([P, W], f32, tag="b")
                        nc.scalar.activation(bT[:], psB[:], AF.Sqrt,
                                             bias=nby[:, ci:ci + 1],
                                             accum_out=st[1][:, col:col + 1])
                        b16 = trd.tile([P, W], bf, tag="b16")
                        nc.vector.tensor_copy(b16[:], bT[:])
                        colsum(cs, b16, 64, ci)
                        t0 = trd.tile([P, W], bf, tag="t")
                        nc.vector.scalar_tensor_tensor(
                            t0[:], aW1[:, bass.ts(ci, W)], MU, bT[:],
                            op0=A.subtract, op1=A.mult,
                            accum_out=st[2][:, col:col + 1])
                    csl = bass.ts(w - 1, 512)
                    nc.vector.tensor_copy(colsave[:, csl], cs[:])
                    nc.sync.dma_start(dcols[:, csl], colsave[:, csl])
                    continue
                for ci in range(CI_N):
                    col = ci * NW + w
                    psA = pp.tile([P, W], f32, tag="ps")
                    psB = pp.tile([P, W], f32, tag="ps")
                    for ps_, ww, mov in ((psA, wx, xm), (psB, wy, ym)):
                        mains(ps_, ww, mov, w, ci)
                    aT = abp.tile([P, W], f32, tag="a")
                    bT = abp.tile([P, W], f32, tag="b")
                    nc.scalar.activation(aT[:], psA[:], AF.Sqrt,
                                         bias=nbx[:, ci:ci + 1],
                                         accum_out=st[0][:, col:col + 1])
                    nc.scalar.activation(bT[:], psB[:], AF.Sqrt,
                                         bias=nby[:, ci:ci + 1],
                                         accum_out=st[1][:, col:col + 1])
                    if do_cs:
                        # bf16 copies feed the column-sum matmuls; the fp32
                        # originals feed the product pass (bf16 rounding
                        # bias is amplified ~1e4x by cancellation in the
                        # final sums, so rs/pab paths must stay fp32). The
                        # copies also decouple ACT from PE's colsum lag.
                        a16 = trd.tile([P, W], bf, tag="a16")
                        b16 = trd.tile([P, W], bf, tag="b16")
                        nc.vector.tensor_copy(a16[:], aT[:])
                        nc.vector.tensor_copy(b16[:], bT[:])
                        colsum(cs, a16, 0, ci)
                        colsum(cs, b16, 64, ci)
                    t0 = trd.tile([P, W], bf, tag="t")
                    nc.vector.scalar_tensor_tensor(
                        t0[:], aT[:], MU, bT[:], op0=A.subtract, op1=A.mult,
                        accum_out=st[2][:, col:col + 1])
                if do_cs:
                    csl = bass.ts(w - 1, 512)
                    nc.vector.tensor_copy(colsave[:, csl], cs[:])
                    nc.sync.dma_start(dcols[:, csl], colsave[:, csl])

            # ── epilogue: ship result ─────────────────────────────────
            nc.sync.dma_start(dst01[0][:], st[0][:])
            nc.scalar.dma_start(dst01[1][:], st[1][:])
            nc.sync.dma_start(dst2[:], st[2][:])
            nc.scalar.dma_start(dout[:], res[:])

    nc.compile()
    return nc


def _get_program(mode: str):
    if mode not in _programs:
        _programs[mode] = _build(mode)
    return _programs[mode]


def make_in_maps(x: np.ndarray, y: np.ndarray, mode: str = "symdr"):
    x = np.asarray(x, np.float32)
    y = np.asarray(y, np.float32)
    xb = x.astype(F8)
    yb = y.astype(F8)

    eye = (np.eye(P) * MU).astype(F8)

    def norms_split(vb, parts=3):
        n64 = (vb.astype(np.float64) ** 2).sum(axis=1)
        rows, rem = [], n64.copy()
        for _ in range(parts):
            r = rem.astype(F8)
            rows.append(r)
            rem = rem - r.astype(np.float64)
        return n64, np.stack(rows).astype(F8)

    nx64, nfx = norms_split(xb)
    ny64, nfy = norms_split(yb)

    xT = np.ascontiguousarray(xb.T)           # [128, 8192]
    yT = np.ascontiguousarray(yb.T)
    xT2 = np.concatenate([xT, xT], axis=1)    # for cheap rotation slicing
    yT2 = np.concatenate([yT, yT], axis=1)
    nfx2 = np.concatenate([nfx, nfx], axis=1)
    nfy2 = np.concatenate([nfy, nfy], axis=1)

    in_maps = []
    for c in range(NCORES):
        o = c * BLK
        im = {
            "xm": np.ascontiguousarray(
                xT2[:, o:o + NWW]).reshape(P, NW, W),
            "ym": np.ascontiguousarray(
                yT2[:, o:o + NWW]).reshape(P, NW, W),
            "nx3": np.ascontiguousarray(nfx2[:, o:o + NWW]),
            "ny3": np.ascontiguousarray(nfy2[:, o:o + NWW]),
            "wx": np.ascontiguousarray(F8(-2.0) * xT2[:, o:o + BLK]),
            "wy": np.ascontiguousarray(F8(-2.0) * yT2[:, o:o + BLK]),
            "nbx": np.ascontiguousarray(
                nx64[o:o + BLK].reshape(CI_N, P).T.astype(np.float32)),
            "nby": np.ascontiguousarray(
                ny64[o:o + BLK].reshape(CI_N, P).T.astype(np.float32)),
            "eye128": eye,
        }
        in_maps.append(im)
    host = {"xb64": xb.astype(np.float64), "yb64": yb.astype(np.float64)}
    return in_maps, host


def finalize(outs, host, mode="symdr", colss=None, st2s=None):
    """Combine per-core outputs -> scalar dcor (host fp64).

    Each core computed local windows 0..4 only. Full sums over the
    symmetric matrices: sum_full = 2*sum_computed - S(w=0) - S(w=4); full
    row sums add mirrored column sums from windows 1..3 of cores bi-1..bi-3.
    Device row sums include the forced diagonal entry sqrt(mu^2) = mu
    (true diag of a distance matrix is 0).
    """
    n = float(N)
    rs_a = np.empty(N, np.float64)
    rs_b = np.empty(N, np.float64)

    # exact Frobenius norms of the quantized-point distance matrices
    def sq_frob(v64):
        s = v64.sum(axis=0)
        return 2.0 * n * (v64 * v64).sum() - 2.0 * np.dot(s, s)

    sq_a = sq_frob(host["xb64"])
    sq_b = sq_frob(host["yb64"])

    # per-row strip sums + per-window totals (for 2*S - S0 - S4)
    tot = np.zeros((2, 3), np.float64)  # [a/b][all, w0, w4]
    for c in range(NCORES):
        for q, rs in ((0, rs_a), (1, rs_b)):
            sq_ = np.asarray(st2s[c][q], np.float64).reshape(P, CI_N, NW)
            rs[c * BLK:(c + 1) * BLK] = sq_.sum(axis=2).T.ravel()
            tot[q] += (sq_.sum(), sq_[:, :, 0].sum(), sq_[:, :, 4].sum())
    # mirrored row-sum contributions from columns of windows 1..3
    VA = np.empty((NCORES, 3, BLK), np.float64)
    VB = np.empty((NCORES, 3, BLK), np.float64)
    for c, cl in enumerate(colss):
        cl = np.asarray(cl, np.float64)
        for wp in range(1, 4):
            sl = slice((wp - 1) * 512, wp * 512)
            VA[c, wp - 1] = np.concatenate([cl[0, sl], cl[32, sl]])
            VB[c, wp - 1] = np.concatenate([cl[64, sl], cl[96, sl]])
    for bi in range(NCORES):
        for wp in range(1, 4):
            rs_a[bi * BLK:(bi + 1) * BLK] += VA[(bi - wp) % NCORES, wp - 1]
            rs_b[bi * BLK:(bi + 1) * BLK] += VB[(bi - wp) % NCORES, wp - 1]
    # pab over the full matrix: 2*computed - S(w0) - S(w4)
    p_all = p_w0 = p_w4 = 0.0
    for s2 in [st2s[c][2] for c in range(NCORES)]:
        s2 = np.asarray(s2, np.float64).reshape(P, CI_N, NW)
        p_all += s2.sum()
        p_w0 += s2[:, :, 0].sum()
        p_w4 += s2[:, :, 4].sum()
    pab = 2.0 * p_all - p_w0 - p_w4
    # unbiased global sums (fp32 accum path; removes forced diag)
    Ra = 2.0 * tot[0, 0] - tot[0, 1] - tot[0, 2] - n * MU
    Rb = 2.0 * tot[1, 0] - tot[1, 1] - tot[1, 2] - n * MU
    sa = rs_a - MU         # per-row (mirror part carries tiny bf16 bias;
    sb = rs_b - MU         # centered dots below are immune to it)
    sat = sa - Ra / n
    sbt = sb - Rb / n
    sum_ab = pab + MU * Rb
    sumAB = sum_ab - 2.0 * np.dot(sat, sbt) / n - Ra * Rb / n**2
    sumAA = sq_a - 2.0 * np.dot(sat, sat) / n - Ra * Ra / n**2
    sumBB = sq_b - 2.0 * np.dot(sbt, sbt) / n - Rb * Rb / n**2
    inv_n2 = 1.0 / (n * n)
    return np.asarray(
        -np.sqrt(sumAB * inv_n2)
        / np.sqrt(np.sqrt(sumAA * inv_n2) * np.sqrt(sumBB * inv_n2)),
        dtype=np.float32)


def run(x, y, mm_mode=None, trace=False, tmpdir=None):
    mode = mm_mode if mm_mode in MODES else DEFAULT_MODE
    nc = _get_program(mode)
    in_maps, host = make_in_maps(x, y, mode)
    res = run_bass_kernel_spmd(nc, in_maps, core_ids=list(range(NCORES)),
                               trace=trace, tmpdir=tmpdir)
    outs = [r["out"] for r in res.results]
    colss = [r["cols"] for r in res.results]
    st2s = [(r["st0o"], r["st1o"], r["st2"]) for r in res.results]
    return finalize(outs, host, mode, colss, st2s), res


def kernel(x, y):
    val, _ = run(x, y)
    return val


# revision 16
# speedup vs baseline: 1.0227x; 1.0227x over previous
"""Distance-correlation (DcorLoss) kernel for 8 trn2 NeuronCores.

Math: for x, y [n=8192, d=128]:
  a = pairwise_dist(x), b = pairwise_dist(y)   (n x n, symmetric, zero diag)
  A = double_center(a), B = double_center(b)
  dcor = -sqrt(sum(A*B)) / sqrt(sqrt(sum(A*A)) * sqrt(sum(B*B)))

Never materialize A/B:
  sum(A o B) = sum(a o b) - 2/n dot(rs_a, rs_b) + sum(a) sum(b) / n^2
and the squared-distance Frobenius norms have a closed form (host, exact):
  sum_ij dist^2 = 2n sum_i |x_i|^2 - 2 |sum_i x_i|^2
so the device only streams: row sums of a and b (ACT accum), column sums
(PE matmul with ones weights), and sum (a - mu) * b (DVE accum). All
combining is host fp64.

Structural tricks:
1. fp8 DoubleRow matmul (K=256 virtual): plane 0 = data rows
   (-2 x_blk^T x gram), plane 1 rows 0..2 = column-norm hi/lo/lo2 splits
   against all-ones weight rows; one MM per 512-col psum half.
2. Symmetry: core c computes only local windows 0..4 (5/8 of the work);
   full sums via 2*computed - S(w0) - S(w4); full row sums add mirrored
   per-column sums of windows 1..3 (PE ones-matmuls at psum partitions
   0/32/64/96), gathered on host.
3. Per-core COLUMN ROTATION keeps the SPMD program identical; the mu^2
   diagonal forcing (sqrt NaN-safety) costs 2 bf16 matmuls on window 0.

Precision: final sums cancel ~1e8 -> ~1e6 (bias amplified ~1e4x), so
products and row-sum accums run on fp32 ACT outputs; only the column-sum
matmuls read bf16 copies; global Ra/Rb come from fp32 accum totals; the
dot products are mean-centered to cancel constant mirror-path bias.

ACT (ScalarE) is the bottleneck: 80 sqrt passes (1024-wide, PSUM->SBUF,
~1181ns pitch) ~= 94.5us; PE (DR mains + colsums) and DVE (products +
bf16 casts) overlap underneath. Measured engine loads: ACT ~101us,
PE ~90us, DVE ~95us effective -- a three-way near-tie, so shifting work
off ACT (DVE reduce ops are 1x-mode, 1219ns) does not pay. The unused
res/dout output path of the original was dropped (small tail saving).
"""

import numpy as np
import ml_dtypes

import concourse.bass as bass
import concourse.tile as tile
from concourse import bacc, mybir
from concourse.bass_utils import run_bass_kernel_spmd

P = 128
N = 8192
NCORES = 8
BLK = N // NCORES
CI_N = BLK // P
W = 1024
JT_N = N // W
MU = 16.0
RES_W = 24

BF16 = ml_dtypes.bfloat16
F8 = ml_dtypes.float8_e4m3

DEFAULT_MODE = "symdr"
MODES = ("base", "dr", "symdr")
NW = 5
CS_ROWS = (0, 32, 64, 96)

_programs = {}

NW_SYM = 5
NCOL = N


def _build(mode: str):
    dt = mybir.dt
    f32 = dt.float32
    bf = dt.bfloat16
    A = mybir.AluOpType
    AF = mybir.ActivationFunctionType

    f8 = dt.float8e4

    nc = bacc.Bacc("TRN2", target_bir_lowering=False, debug=False,
                   num_devices=NCORES)

    dxm = nc.dram_tensor("xm", [P, JT_N, 2, W], f8,
                         kind="ExternalInput").ap()
    dym = nc.dram_tensor("ym", [P, JT_N, 2, W], f8,
                         kind="ExternalInput").ap()
    dwx = nc.dram_tensor("wx", [P, 2, BLK], f8, kind="ExternalInput").ap()
    dwy = nc.dram_tensor("wy", [P, 2, BLK], f8, kind="ExternalInput").ap()
    dnbx = nc.dram_tensor("nbx", [P, CI_N], f32, kind="ExternalInput").ap()
    dnby = nc.dram_tensor("nby", [P, CI_N], f32, kind="ExternalInput").ap()
    deye = nc.dram_tensor("eye128", [P, P], bf, kind="ExternalInput").ap()
    dew = nc.dram_tensor("eyewide", [P, 4 * 512], bf, kind="ExternalInput").ap()
    dout = nc.dram_tensor("out", [P, RES_W], f32, kind="ExternalOutput").ap()
    dcols = nc.dram_tensor("cols", [P, 3 * 512], f32,
                           kind="ExternalOutput").ap()
    dst01 = [nc.dram_tensor(f"st{q}o", [P, CI_N * NW], f32,
                            kind="ExternalOutput").ap() for q in range(2)]
    dst2 = nc.dram_tensor("st2", [P, CI_N * NW], f32,
                          kind="ExternalOutput").ap()
    n_w = NW

    with tile.TileContext(nc) as tc:
        with tc.tile_pool(name="const", bufs=1) as cp, \
             tc.tile_pool(name="psum", bufs=3, space="PSUM") as pp, \
             tc.tile_pool(name="cspsum", bufs=2, space="PSUM") as csp, \
             tc.tile_pool(name="ab", bufs=3) as abp, \
             tc.tile_pool(name="trd", bufs=2) as trd:

            xm = cp.tile([P, 2, N], f8, tag="xm")
            ym = cp.tile([P, 2, N], f8, tag="ym")
            wx = cp.tile([P, 2, BLK], f8, tag="wx")
            wy = cp.tile([P, 2, BLK], f8, tag="wy")
            nbx = cp.tile([P, CI_N], f32, tag="nbx")
            nby = cp.tile([P, CI_N], f32, tag="nby")
            eye128 = cp.tile([P, P], bf, tag="eye128")
            eyew = cp.tile([P, 4 * 512], bf, tag="eyew")
            ones2 = cp.tile([2, P], bf, tag="ones2")
            nc.vector.memset(ones2[:], 1.0)

            st = [cp.tile([P, CI_N * n_w], f32, tag=f"st{q}", name=f"st{q}")
                  for q in range(3)]
            onesP = cp.tile([P, 1], bf, tag="onesP")
            nc.vector.memset(onesP[:], 1.0)
            colsave = cp.tile([P, 3 * 512], f32, tag="colsave")

            wur = cp.tile([2, 512], bf, tag="wur")
            nc.vector.memset(wur[:], 0.0)
            tldu = cp.tile([1, 8], f32, tag="tldu")
            tlda = cp.tile([1, 1], f32, tag="tlda")
            tldb = cp.tile([1, 1], f32, tag="tldb")
            nc.vector.memset(tldb[:], 0.0)
            nc.scalar.activation(tldu[:], wur[0:1, 0:8], AF.Sqrt,
                                 bias=tldb[:], accum_out=tlda[:])

            w_first = 1
            sl0 = bass.ts(w_first, W)
            nc.sync.dma_start(wx[:], dwx[:])
            nc.scalar.dma_start(wy[:], dwy[:])
            for lo, hi in ((0, 64), (64, 128)):
                eng = nc.sync if lo == 0 else nc.scalar
                eng.dma_start(xm[lo:hi, :, sl0], dxm[lo:hi, w_first])
            for lo, hi in ((0, 64), (64, 128)):
                eng = nc.scalar if lo == 0 else nc.sync
                eng.dma_start(ym[lo:hi, :, sl0], dym[lo:hi, w_first])
            nc.sync.dma_start(nbx[:], dnbx[:])
            nc.scalar.dma_start(nby[:], dnby[:])
            nc.sync.dma_start(eye128[:], deye[:])
            order = [2, 3, 4, 0]
            for w in order:
                sl = bass.ts(w, W)
                nc.sync.dma_start(xm[:, :, sl], dxm[:, w])
                nc.sync.dma_start(ym[:, :, sl], dym[:, w])
            nc.sync.dma_start(eyew[:], dew[:])

            w_order = [1, 2, 3, 4, 0]
            for w in w_order:
                do_cs = 1 <= w <= 3
                if do_cs:
                    cs = csp.tile([P, 512], f32, tag="cs")
                for ci in range(CI_N):
                    col = ci * n_w + w
                    h_diag = ci // 4
                    psA = pp.tile([P, W], f32, tag="ps")
                    psB = pp.tile([P, W], f32, tag="ps")
                    for ps_, ww, mov in ((psA, wx, xm), (psB, wy, ym)):
                        for h in range(2):
                            last = not (w == 0 and h == h_diag)
                            nc.tensor.matmul(
                                ps_[:, bass.ds(h * 512, 512)],
                                ww[:, :, bass.ts(ci, P)],
                                mov[:, :, bass.ds(w * W + h * 512, 512)],
                                start=True, stop=last,
                                perf_mode=mybir.MatmulPerfMode.DoubleRow)
                        if w == 0:
                            nc.tensor.matmul(
                                ps_[:, bass.ds(h_diag * 512, 512)],
                                eye128[:],
                                eyew[:, bass.ts(ci % 4, 512)],
                                start=False, stop=True)

                    aT = abp.tile([P, W], f32, tag="a")
                    bT = abp.tile([P, W], f32, tag="b")
                    nc.scalar.activation(aT[:], psA[:], AF.Sqrt,
                                         bias=nbx[:, ci:ci + 1],
                                         accum_out=st[0][:, col:col + 1])
                    nc.scalar.activation(bT[:], psB[:], AF.Sqrt,
                                         bias=nby[:, ci:ci + 1],
                                         accum_out=st[1][:, col:col + 1])
                    if do_cs:
                        a16 = trd.tile([P, W], bf, tag="a16")
                        b16 = trd.tile([P, W], bf, tag="b16")
                        nc.vector.tensor_copy(a16[:], aT[:])
                        nc.vector.tensor_copy(b16[:], bT[:])
                        for r, t_ in ((0, a16), (64, b16)):
                            for h in range(2):
                                nc.tensor.matmul(
                                    cs[r + 32 * h:r + 32 * h + 1, 0:512],
                                    onesP[:],
                                    t_[:, bass.ds(h * 512, 512)],
                                    start=(ci == 0), stop=(ci == CI_N - 1),
                                    tile_position=(0, r + 32 * h))
                    t0 = trd.tile([P, W], bf, tag="t")
                    nc.vector.scalar_tensor_tensor(
                        t0[:], aT[:], MU, bT[:], op0=A.subtract, op1=A.mult,
                        accum_out=st[2][:, col:col + 1])
                if do_cs:
                    csl = bass.ts(w - 1, 512)
                    nc.vector.tensor_copy(colsave[:, csl], cs[:])
                    nc.sync.dma_start(dcols[:, csl], colsave[:, csl])

            nc.sync.dma_start(dst01[0][:], st[0][:])
            nc.scalar.dma_start(dst01[1][:], st[1][:])
            nc.sync.dma_start(dst2[:], st[2][:])

    nc.compile()
    return nc


def _get_program(mode: str):
    if mode not in _programs:
        _programs[mode] = _build(mode)
    return _programs[mode]


def make_in_maps(x: np.ndarray, y: np.ndarray, mode: str = "symdr"):
    x = np.asarray(x, np.float32)
    y = np.asarray(y, np.float32)
    QD = F8
    xb = x.astype(QD)
    yb = y.astype(QD)

    eye = (np.eye(P) * MU).astype(BF16)
    ew = np.zeros((P, 4 * 512), BF16)
    for k in range(4):
        for p in range(P):
            ew[p, k * 512 + k * P + p] = BF16(MU)

    def norms_split(vb, parts):
        n64 = (vb.astype(np.float64) ** 2).sum(axis=1)
        rows, rem = [], n64.copy()
        for _ in range(parts):
            r = rem.astype(QD)
            rows.append(r)
            rem = rem - r.astype(np.float64)
        return n64, np.stack(rows).astype(QD)

    nx64, nfx = norms_split(xb, 3)
    ny64, nfy = norms_split(yb, 3)

    xT = np.ascontiguousarray(xb.T)
    yT = np.ascontiguousarray(yb.T)
    xT2 = np.concatenate([xT, xT], axis=1)
    yT2 = np.concatenate([yT, yT], axis=1)
    nfx2 = np.concatenate([nfx, nfx], axis=1)
    nfy2 = np.concatenate([nfy, nfy], axis=1)

    in_maps = []
    for c in range(NCORES):
        o = c * BLK
        xTr = np.ascontiguousarray(xT2[:, o:o + N])
        yTr = np.ascontiguousarray(yT2[:, o:o + N])
        im = {
            "nbx": np.ascontiguousarray(
                nx64[o:o + BLK].reshape(CI_N, P).T.astype(np.float32)),
            "nby": np.ascontiguousarray(
                ny64[o:o + BLK].reshape(CI_N, P).T.astype(np.float32)),
            "eye128": eye,
            "eyewide": ew,
        }

        def moving(vT, nf2):
            m = np.zeros((P, 2, N), QD)
            m[:, 0, :] = vT
            m[0:3, 1, :] = nf2[:, o:o + N]
            return np.ascontiguousarray(
                m.reshape(P, 2, JT_N, W).transpose(0, 2, 1, 3))

        def weights(vT):
            ww = np.zeros((P, 2, BLK), QD)
            ww[:, 0, :] = QD(-2.0) * vT[:, 0:BLK]
            ww[0:3, 1, :] = QD(1.0)
            return ww

        im.update({"xm": moving(xTr, nfx2), "ym": moving(yTr, nfy2),
                   "wx": weights(xTr), "wy": weights(yTr)})
        in_maps.append(im)
    host = {"xb64": xb.astype(np.float64), "yb64": yb.astype(np.float64)}
    return in_maps, host


def finalize(outs, host, mode="symdr", colss=None, st2s=None):
    """Combine per-core outputs -> scalar dcor (host fp64).

    Each core computed local windows 0..4 only. Full sums over the
    symmetric matrices: sum_full = 2*sum_computed - S(w=0) - S(w=4); full
    row sums add mirrored column sums from windows 1..3 of cores bi-1..bi-3.
    Device row sums include the forced diagonal entry sqrt(mu^2) = mu
    (true diag of a distance matrix is 0).
    """
    n = float(N)
    rs_a = np.empty(N, np.float64)
    rs_b = np.empty(N, np.float64)

    # exact Frobenius norms of the quantized-point distance matrices
    def sq_frob(v64):
        s = v64.sum(axis=0)
        return 2.0 * n * (v64 * v64).sum() - 2.0 * np.dot(s, s)

    sq_a = sq_frob(host["xb64"])
    sq_b = sq_frob(host["yb64"])

    # per-row strip sums + per-window totals (for 2*S - S0 - S4)
    tot = np.zeros((2, 3), np.float64)  # [a/b][all, w0, w4]
    for c in range(NCORES):
        for q, rs in ((0, rs_a), (1, rs_b)):
            sq_ = np.asarray(st2s[c][q], np.float64).reshape(P, CI_N, NW)
            rs[c * BLK:(c + 1) * BLK] = sq_.sum(axis=2).T.ravel()
            tot[q] += (sq_.sum(), sq_[:, :, 0].sum(), sq_[:, :, 4].sum())
    # mirrored row-sum contributions from columns of windows 1..3
    VA = np.empty((NCORES, 3, BLK), np.float64)
    VB = np.empty((NCORES, 3, BLK), np.float64)
    for c, cl in enumerate(colss):
        cl = np.asarray(cl, np.float64)
        for wp in range(1, 4):
            sl = slice((wp - 1) * 512, wp * 512)
            VA[c, wp - 1] = np.concatenate([cl[0, sl], cl[32, sl]])
            VB[c, wp - 1] = np.concatenate([cl[64, sl], cl[96, sl]])
    for bi in range(NCORES):
        for wp in range(1, 4):
            rs_a[bi * BLK:(bi + 1) * BLK] += VA[(bi - wp) % NCORES, wp - 1]
            rs_b[bi * BLK:(bi + 1) * BLK] += VB[(bi - wp) % NCORES, wp - 1]
    # pab over the full matrix: 2*computed - S(w0) - S(w4)
    p_all = p_w0 = p_w4 = 0.0
    for s2 in [st2s[c][2] for c in range(NCORES)]:
        s2 = np.asarray(s2, np.float64).reshape(P, CI_N, NW)
        p_all += s2.sum()
        p_w0 += s2[:, :, 0].sum()
        p_w4 += s2[:, :, 4].sum()
    pab = 2.0 * p_all - p_w0 - p_w4
    # unbiased global sums (fp32 accum path; removes forced diag)
    Ra = 2.0 * tot[0, 0] - tot[0, 1] - tot[0, 2] - n * MU
    Rb = 2.0 * tot[1, 0] - tot[1, 1] - tot[1, 2] - n * MU
    sa = rs_a - MU         # per-row (mirror part carries tiny bf16 bias;
    sb = rs_b - MU         # centered dots below are immune to it)
    sat = sa - Ra / n
    sbt = sb - Rb / n
    sum_ab = pab + MU * Rb
    sumAB = sum_ab - 2.0 * np.dot(sat, sbt) / n - Ra * Rb / n**2
    sumAA = sq_a - 2.0 * np.dot(sat, sat) / n - Ra * Ra / n**2
    sumBB = sq_b - 2.0 * np.dot(sbt, sbt) / n - Rb * Rb / n**2
    inv_n2 = 1.0 / (n * n)
    return np.asarray(
        -np.sqrt(sumAB * inv_n2)
        / np.sqrt(np.sqrt(sumAA * inv_n2) * np.sqrt(sumBB * inv_n2)),
        dtype=np.float32)



def run(x, y, mm_mode=None, trace=False, tmpdir=None):
    mode = "symdr"
    nc = _get_program(mode)
    in_maps, host = make_in_maps(x, y, mode)
    res = run_bass_kernel_spmd(nc, in_maps, core_ids=list(range(NCORES)),
                               trace=trace, tmpdir=tmpdir)
    outs = [r["out"] for r in res.results]
    colss = [r["cols"] for r in res.results]
    st2s = [(r["st0o"], r["st1o"], r["st2"]) for r in res.results]
    return finalize(outs, host, mode, colss, st2s), res


def kernel(x, y):
    val, _ = run(x, y)
    return val
